# revision 40
# baseline (speedup 1.0000x reference)
"""Causal self-attention (B=4, T=2048, C=1024, H=16) on 8 TRN2 NeuronCores.

Sharding: core = (batch b, head-group hg). Data parallel over B (4), tensor
parallel over heads (2 groups of 8). Each core computes a partial output
projection for its 8 heads; the host sums the two partials per batch.

v4 design (over v3):
  - q/k projections in fp8e4 DoubleRow (K=256 per matmul, 2 fp8 MACs per
    cell per cycle): host ships xT8 + 32*wq/32*wk in fp8; the 32x weight
    scale dodges e4m3 denormals and is folded into the exp scale
    (SCALE/1024). Softmax damps the quantization so output error stays
    well under budget. v projection / scores / pv / proj stay bf16-f16.
  - lag-2 pv pipeline: pv(si) is emitted two scores-steps after
    scores(si), so the ~1us exp latency hides behind real PE work
    instead of stalling the in-order PE queue (v3 used lag-1).
  - diag causal masks via gpsimd.affine_select directly on pT (frees
    DVE, keeps the exp->mask->pv chain off the busy vector queue).
  - per-mt batched normalization: one [1,1024] reciprocal + one
    [64,1024] partition_broadcast for the even+odd head pair, emitted
    dn-first so the PSUM accumulator banks free early.
  - fillers rebalanced by estimated PE-time: qkv(nt+1) + a slice of
    pending projs sized so no window goes ACT-idle; remaining projs ride
    the last window, and the 4 final projs pre-emit their kt0..2
    accumulation chains during the last norm chain so only kt3 +
    evacuation trails the last attention step.
  - tail evacuations alternate DVE/ACT and out-DMAs alternate
    sync/vector queues.
"""
import numpy as np
import ml_dtypes
from contextlib import ExitStack

import concourse.bass as bass
import concourse.mybir as mybir
import concourse.tile as tile
from concourse import bacc
from concourse.bass_utils import run_bass_kernel_spmd

F32 = mybir.dt.float32
BF16 = mybir.dt.bfloat16
F16 = mybir.dt.float16
FP8 = mybir.dt.float8e4
AF = mybir.ActivationFunctionType
DR = mybir.MatmulPerfMode.DoubleRow

T = 2048
C = 1024
H_PER_CORE = 8          # heads per core
D = 64                  # head dim
GC = H_PER_CORE * D     # 512 channels per head-group
SCALE = 1.0 / 32.0 / 1024.0   # C**-0.5, divided by the 32x*32x fp8 w scale
N_CORES = 8
NT = T // 128           # 16 t-tiles
NC_ = C // 128          # 8 c-tiles
NQSB = T // 512         # 4 q superblocks


def build(nc):
    xT8_d = nc.dram_tensor("xT8", [C, T], FP8, kind="ExternalInput").ap()
    xT_d = nc.dram_tensor("xT", [C, T], BF16, kind="ExternalInput").ap()
    wq_d = nc.dram_tensor("wq", [C, GC], FP8, kind="ExternalInput").ap()
    wk_d = nc.dram_tensor("wk", [C, GC], FP8, kind="ExternalInput").ap()
    wv_d = nc.dram_tensor("wv", [C, GC], BF16, kind="ExternalInput").ap()
    wp_d = nc.dram_tensor("wp", [GC, C], BF16, kind="ExternalInput").ap()
    out_d = nc.dram_tensor("out", [T, C], BF16, kind="ExternalOutput").ap()

    with tile.TileContext(nc) as tc, ExitStack() as ctx:
        const = ctx.enter_context(tc.tile_pool(name="const", bufs=1))
        persist = ctx.enter_context(tc.tile_pool(name="persist", bufs=1))
        pT_pool = ctx.enter_context(tc.tile_pool(name="pT", bufs=12))
        ySt_pool = ctx.enter_context(tc.tile_pool(name="ySt", bufs=4))
        rc_pool = ctx.enter_context(tc.tile_pool(name="rc", bufs=2))
        rb_pool = ctx.enter_context(tc.tile_pool(name="rb", bufs=3))
        so_pool = ctx.enter_context(tc.tile_pool(name="so", bufs=3))
        psS = ctx.enter_context(tc.tile_pool(name="psS", bufs=3, space="PSUM"))
        psY = ctx.enter_context(tc.tile_pool(name="psY", bufs=2, space="PSUM"))

        warm_sb = const.tile([128, 512], F16)
        nc.gpsimd.memset(warm_sb[:], 0.0)
        ones64 = const.tile([1, 64], F32)
        nc.gpsimd.memset(ones64[:], 1.0)
        # tri_mask[k, j] = 1.0 if k <= j else 0.0 (for the DVE mask path)
        tri_mask = const.tile([128, 128], F16)
        nc.gpsimd.memset(tri_mask[:], 1.0)
        nc.gpsimd.affine_select(
            out=tri_mask[:], in_=tri_mask[:],
            compare_op=mybir.AluOpType.is_ge, fill=0.0, base=0,
            pattern=[[1, 128]], channel_multiplier=-1,
        )

        # persistent SBUF
        xT8_sb = persist.tile([128, NC_, T], FP8)
        xT_sb = persist.tile([128, NC_, T], BF16)
        qT_sb = persist.tile([128, 4, T], BF16)
        kT_sb = persist.tile([128, 4, T], BF16)
        v_aug = persist.tile([128, H_PER_CORE, NT, 65], F16)
        yT_sb = persist.tile([128, 4, T], BF16)
        wq_sb = persist.tile([128, NC_, GC], FP8)
        wk_sb = persist.tile([128, NC_, GC], FP8)
        wv_sb = persist.tile([128, NC_, GC], BF16)
        wp_sb = persist.tile([128, 4, C], BF16)

        nc.gpsimd.memset(v_aug[:, :, :, 64], 1.0)

        # preload the Exp activation table first thing on the scalar queue,
        # so the first real exp doesn't eat the ~2.7us ACT_TABLE_LOAD stall
        dummy_act = const.tile([1, 32], F16)
        nc.scalar.activation(dummy_act[:], warm_sb[0:1, 0:32],
                             AF.Exp, bias=0.0, scale=1.0)

        # ---- PE warmup: back-to-back matmuls keep the HAM clock gate open
        # while the first-wave DMAs land (~12us) ----
        warm = psS.tile([128, 1024], F32, tag="wide")
        for i in range(20):
            nc.tensor.matmul(warm[:, 0:512], warm_sb[:, 0:128], warm_sb[:],
                             start=True, stop=True)

        # ---- DMAs, first-needed-first, plain 2D slices (hardware DGE; a
        # rearranged 3D form goes through slow software descriptor-gen).
        # First wave, split across all three queues: wq8 / wk8 / xT8(0).
        for ct in range(NC_):
            nc.sync.dma_start(wq_sb[:, ct, :], wq_d[ct * 128:(ct + 1) * 128, :])
        for ct in range(NC_):
            nc.scalar.dma_start(wk_sb[:, ct, :], wk_d[ct * 128:(ct + 1) * 128, :])
        for ct in range(NC_):
            nc.gpsimd.dma_start(
                xT8_sb[:, ct, 0:512], xT8_d[ct * 128:(ct + 1) * 128, 0:512])
        # second wave: v(0) inputs + later qk chunks.
        for ct in range(NC_):
            nc.gpsimd.dma_start(
                xT_sb[:, ct, 0:512], xT_d[ct * 128:(ct + 1) * 128, 0:512])
        for ct in range(NC_):
            nc.scalar.dma_start(wv_sb[:, ct, :], wv_d[ct * 128:(ct + 1) * 128, :])
        # later chunks in need-order on the idle sync queue (keeps the
        # gpsimd queue clear for the nt=0 diag masks)
        def sync_x8(ntd):
            nsp = slice(ntd * 512, (ntd + 1) * 512)
            for ct in range(NC_):
                nc.sync.dma_start(
                    xT8_sb[:, ct, nsp], xT8_d[ct * 128:(ct + 1) * 128, nsp])

        def sync_xb(ntd):
            nsp = slice(ntd * 512, (ntd + 1) * 512)
            for ct in range(NC_):
                nc.sync.dma_start(
                    xT_sb[:, ct, nsp], xT_d[ct * 128:(ct + 1) * 128, nsp])

        sync_x8(1)
        sync_xb(1)
        sync_x8(2)
        sync_x8(3)
        sync_xb(2)
        sync_xb(3)
        for kt in range(4):
            nc.scalar.dma_start(wp_sb[:, kt, :], wp_d[kt * 128:(kt + 1) * 128, :])

        # ---- emit helpers ----
        def emit_qk(nt, mt):
            nsp = slice(nt * 512, (nt + 1) * 512)
            msp = slice(mt * 128, (mt + 1) * 128)
            wide = psS.tile([128, 1024], F32, tag="wide")
            for c2 in range(NC_ // 2):
                nc.tensor.matmul(
                    wide[:, 0:512],
                    wq_sb[:, 2 * c2:2 * c2 + 2, msp],
                    xT8_sb[:, 2 * c2:2 * c2 + 2, nsp],
                    start=(c2 == 0), stop=(c2 == NC_ // 2 - 1), perf_mode=DR)
            for c2 in range(NC_ // 2):
                nc.tensor.matmul(
                    wide[:, 512:1024],
                    wk_sb[:, 2 * c2:2 * c2 + 2, msp],
                    xT8_sb[:, 2 * c2:2 * c2 + 2, nsp],
                    start=(c2 == 0), stop=(c2 == NC_ // 2 - 1), perf_mode=DR)
            nc.vector.tensor_copy(qT_sb[:, mt, nsp], wide[:, 0:512])
            nc.vector.tensor_copy(kT_sb[:, mt, nsp], wide[:, 512:1024])

        def emit_v(nt, tp):
            wide = psS.tile([128, 1024], F32, tag="wide")
            for i in range(2):
                tt = 4 * nt + 2 * tp + i
                for ct in range(NC_):
                    nc.tensor.matmul(
                        wide[:, i * 512:(i + 1) * 512],
                        xT_sb[:, ct, tt * 128:(tt + 1) * 128],
                        wv_sb[:, ct, :],
                        start=(ct == 0), stop=(ct == NC_ - 1))
                nc.vector.tensor_copy(
                    v_aug[:, :, tt, 0:64],
                    wide[:, i * 512:(i + 1) * 512].rearrange(
                        "p (h d) -> p h d", h=H_PER_CORE))

        def emit_proj(tt, tail=False):
            wide = psS.tile([128, 1024], F32, tag="wide")
            emit_proj_mm(wide, tt, range(4))
            emit_proj_out(wide, tt, tail)

        def emit_proj_mm(wide, tt, kts):
            tsp = slice(tt * 128, (tt + 1) * 128)
            for n2 in range(2):
                for kt in kts:
                    nc.tensor.matmul(
                        wide[:, n2 * 512:(n2 + 1) * 512],
                        yT_sb[:, kt, tsp],
                        wp_sb[:, kt, n2 * 512:(n2 + 1) * 512],
                        start=(kt == 0), stop=(kt == 3))

        def emit_proj_out(wide, tt, tail=False):
            tsp = slice(tt * 128, (tt + 1) * 128)
            so = so_pool.tile([128, 1024], BF16)
            for h in range(2):
                hs = slice(h * 512, (h + 1) * 512)
                cp = nc.scalar.copy if (tail and h == 1) else nc.vector.tensor_copy
                cp(so[:, hs], wide[:, hs])
                dq = nc.scalar if (tail and h == 1) else nc.sync
                dq.dma_start(out_d[tsp, hs], so[:, hs])

        def emit_scores(nt, mt, kb):
            lo = max(0, kb * 128 - nt * 512)
            ksp = slice(kb * 128, (kb + 1) * 128)
            qsl = slice(nt * 512 + lo, (nt + 1) * 512)
            wide = psS.tile([128, 1024], F32, tag="wide")
            nc.tensor.matmul(
                wide[:, lo:512],
                kT_sb[0:64, mt, ksp], qT_sb[0:64, mt, qsl],
                start=True, stop=True)
            nc.tensor.matmul(
                wide[:, 512 + lo:1024],
                kT_sb[64:128, mt, ksp], qT_sb[64:128, mt, qsl],
                start=True, stop=True)
            pT = pT_pool.tile([128, 1024], F16, tag="pT")
            if lo == 0:
                nc.scalar.activation(
                    pT[:, 0:1024], wide[:, 0:1024],
                    AF.Exp, bias=0.0, scale=SCALE)
            else:
                w2 = wide[:].rearrange("p (h q) -> p h q", h=2)
                p2 = pT[:].rearrange("p (h q) -> p h q", h=2)
                nc.scalar.activation(
                    p2[:, :, lo:512], w2[:, :, lo:512],
                    AF.Exp, bias=0.0, scale=SCALE)
            if kb >= 4 * nt:  # diagonal block: causal mask within block
                # DVE tri-mask muls are cheap (~200ns) and keep the mask off
                # the gpsimd queue, whose broadcasts would stall the pv chain
                nc.vector.tensor_mul(
                    pT[:, lo:lo + 128], pT[:, lo:lo + 128], tri_mask[:])
                nc.vector.tensor_mul(
                    pT[:, 512 + lo:512 + lo + 128],
                    pT[:, 512 + lo:512 + lo + 128], tri_mask[:])
            return (pT, lo)

        def emit_pv(mt, kb, nkb, yTe, yTo, pT, lo):
            nc.tensor.matmul(
                yTe[:, lo:512], v_aug[:, 2 * mt, kb, :], pT[:, lo:512],
                start=(kb == 0), stop=(kb == nkb - 1))
            nc.tensor.matmul(
                yTo[:, lo:512], v_aug[:, 2 * mt + 1, kb, :],
                pT[:, 512 + lo:1024],
                start=(kb == 0), stop=(kb == nkb - 1))

        pend_mul = []  # deferred normalization multiplies (phase B)

        def emit_norm(nt, mt, yTe, yTo, last=False):
            nsp = slice(nt * 512, (nt + 1) * 512)
            # phase A: dn first (recip chain starts), then stage out of PSUM
            # to free the accumulator banks for the next mt pair
            dn = rc_pool.tile([1, 1024], F32, tag="dn")
            nc.vector.tensor_copy(dn[:, 0:512], yTe[64:65, :])
            nc.vector.tensor_copy(dn[:, 512:1024], yTo[64:65, :])
            ySe = ySt_pool.tile([65, 512], F32, tag="ySt")
            nc.vector.tensor_copy(ySe[:], yTe[:])
            ySo = ySt_pool.tile([65, 512], F32, tag="ySt")
            nc.vector.tensor_copy(ySo[:], yTo[:])
            recip = rc_pool.tile([1, 1024], F32, tag="recip")
            nc.vector.reciprocal_approx_fast(recip[:], dn[:])
            if last:
                # tail fast path: broadcast the reciprocal via two K=1
                # matmuls (the gpsimd broadcast is ~1.8us; PE is idle-ish
                # here) and normalize immediately
                rbc_e = psY.tile([64, 512], F32, tag="yT", name="rbc_e")
                rbc_o = psY.tile([64, 512], F32, tag="yT", name="rbc_o")
                nc.tensor.matmul(rbc_e[:], ones64[:], recip[:, 0:512],
                                 start=True, stop=True)
                nc.tensor.matmul(rbc_o[:], ones64[:], recip[:, 512:1024],
                                 start=True, stop=True)
                nc.vector.tensor_mul(
                    yT_sb[0:64, mt, nsp], ySe[0:64, :], rbc_e[:])
                nc.vector.tensor_mul(
                    yT_sb[64:128, mt, nsp], ySo[0:64, :], rbc_o[:])
                return
            # phases B (broadcast) and C (muls) are deferred: emitted now
            # they would head-of-line block the gpsimd/DVE queues while
            # waiting on the recip chain
            pend_mul.append((nt, mt, ySe, ySo, recip, None))

        def emit_norm_bcast():
            pnt, pmt, ySe, ySo, recip, _ = pend_mul[0]
            rbc = rb_pool.tile([64, 1024], F32)
            nc.gpsimd.partition_broadcast(rbc[:], recip[:])
            pend_mul[0] = (pnt, pmt, ySe, ySo, recip, rbc)

        def emit_norm_muls():
            pnt, pmt, ySe, ySo, recip, rbc = pend_mul.pop(0)
            if rbc is None:
                rbc = rb_pool.tile([64, 1024], F32)
                nc.gpsimd.partition_broadcast(rbc[:], recip[:])
            nsp = slice(pnt * 512, (pnt + 1) * 512)
            nc.vector.tensor_mul(
                yT_sb[0:64, pmt, nsp], ySe[0:64, :], rbc[:, 0:512])
            nc.vector.tensor_mul(
                yT_sb[64:128, pmt, nsp], ySo[0:64, :], rbc[:, 512:1024])

        # ---- filler plumbing: spread units by estimated PE-time ----
        def unit_cost(u):
            if u[0] == "qk":
                return 1950
            if u[0] == "v":
                return 3450
            return 1750  # proj

        def step_cost(nt, kb):
            lo = max(0, kb * 128 - nt * 512)
            return (512 - lo) * 3 * 5 // 12  # ns, ~3 streams @2.4GHz

        def run_unit(u):
            if u[0] == "qk":
                emit_qk(u[1], u[2])
            elif u[0] == "v":
                emit_v(u[1], u[2])
            else:
                emit_proj(u[1])

        # ---- main emission ----
        for mt in range(4):
            emit_qk(0, mt)
        for tp in range(2):
            emit_v(0, tp)

        LAG = 3
        BC_AGE = 3           # flushes between norm phase A and its broadcast
        MUL_AGE = 6          # flushes between norm phase A and its muls
        pend = []            # (nt, mt, kb, nkb, pT, lo) awaiting pv emission
        yT_pair = {}         # mt -> (yTe, yTo) PSUM accumulators
        flush_idx = [0]
        mul_stamp = []       # flush_idx at which each pend_mul was created

        def flush_one():
            flush_idx[0] += 1
            pnt, pmt, pkb, pnkb, ppT, plo = pend.pop(0)
            if pkb == 0:
                yTe = psY.tile([65, 512], F32, tag="yT")
                yTo = psY.tile([65, 512], F32, tag="yT")
                yT_pair[pmt] = (yTe, yTo)
            emit_pv(pmt, pkb, pnkb, *yT_pair[pmt], ppT, plo)
            if pkb == pnkb - 1:
                last = (pnt == NQSB - 1 and pmt == 3)
                emit_norm(pnt, pmt, *yT_pair[pmt], last=last)
                if not last:
                    mul_stamp.append(flush_idx[0])
            if pend_mul and pend_mul[0][5] is None \
                    and flush_idx[0] - mul_stamp[0] >= BC_AGE:
                emit_norm_bcast()
            while pend_mul and flush_idx[0] - mul_stamp[0] >= MUL_AGE:
                emit_norm_muls()
                mul_stamp.pop(0)

        # which proj tts ride as fillers in each window (rest go to tail):
        # w3 absorbs more projs so it isn't ACT-paced (its own attention
        # steps average just under the ~1us exp per step)
        proj_fill = {1: [0, 1], 2: [2, 3], 3: [4, 5, 6, 7, 8, 9, 10, 11]}

        for nt in range(NQSB):
            nkb = 4 * (nt + 1)
            fillers = []
            if nt + 1 < NQSB:
                for mt in range(4):
                    fillers.append(("qk", nt + 1, mt))
                for tp in range(2):
                    fillers.append(("v", nt + 1, tp))
            for tt in proj_fill.get(nt, []):
                fillers.append(("proj", tt, None))

            steps = [(mt, kb) for mt in range(4) for kb in range(nkb)]
            total_pe = sum(step_cost(nt, kb) for mt, kb in steps)
            total_fill = sum(unit_cost(u) for u in fillers)
            # assign each filler the step index where its fraction of
            # filler-time matches the fraction of step-time elapsed
            fill_at = {}
            acc = 0
            fi = 0
            facc = 0
            for si, (mt, kb) in enumerate(steps):
                acc += step_cost(nt, kb)
                while fi < len(fillers) and \
                        (facc + unit_cost(fillers[fi]) / 2) * total_pe \
                        <= acc * total_fill:
                    fill_at.setdefault(si, []).append(fillers[fi])
                    facc += unit_cost(fillers[fi])
                    fi += 1
            while fi < len(fillers):
                fill_at.setdefault(len(steps) - 1, []).append(fillers[fi])
                fi += 1

            for si, (mt, kb) in enumerate(steps):
                pT, lo = emit_scores(nt, mt, kb)
                for u in fill_at.get(si, []):
                    run_unit(u)
                pend.append((nt, mt, kb, nkb, pT, lo))
                if len(pend) > LAG:
                    flush_one()

        # ---- tail: pre-emit kt0..2 proj chains for the last 4 tts while
        # the final pv/norm chain drains, then finish kt3 + evacuate ----
        tail_tts = [12, 13, 14, 15]
        tail_wides = {}
        # muls for nt=3 mts 0..2 were emitted in-loop by age; drain any
        # stragglers before the proj chains that read them
        while pend_mul:
            emit_norm_muls()
            mul_stamp.pop(0)
        # tt12's kt0..2 chain fills the PE while the last pvs' exps drain;
        # tt13/tt14 chains land between the last pv and the recip so the
        # normalization latency hides behind real matmuls
        tail_wides[12] = psS.tile([128, 1024], F32, tag="wide", name="tw12")
        emit_proj_mm(tail_wides[12], 12, range(3))
        while pend:
            flush_one()
        for tt in tail_tts[1:3]:
            tail_wides[tt] = psS.tile([128, 1024], F32, tag="wide", name=f"tw{tt}")
            emit_proj_mm(tail_wides[tt], tt, range(3))
        dqs = [nc.sync, nc.scalar, nc.gpsimd]
        for ti, tt in enumerate(tail_tts[:3]):
            wide = tail_wides[tt]
            tsp = slice(tt * 128, (tt + 1) * 128)
            emit_proj_mm(wide, tt, [3])
            so = so_pool.tile([128, 1024], BF16, name="so")
            for h in range(2):
                hs = slice(h * 512, (h + 1) * 512)
                cp = nc.scalar.copy if h == 1 else nc.vector.tensor_copy
                cp(so[:, hs], wide[:, hs])
                dqs[(2 * ti + h) % 3].dma_start(out_d[tsp, hs], so[:, hs])
        # tt15 runs in the freed psY banks so it needn't wait for a wide
        # to be evacuated
        p15a = psY.tile([128, 512], F32, tag="yT", name="p15a")
        p15b = psY.tile([128, 512], F32, tag="yT", name="p15b")
        tsp15 = slice(15 * 128, 16 * 128)
        so15 = so_pool.tile([128, 1024], BF16, name="so")
        for n2, pp in enumerate((p15a, p15b)):
            for kt in range(4):
                nc.tensor.matmul(
                    pp[:], yT_sb[:, kt, tsp15],
                    wp_sb[:, kt, n2 * 512:(n2 + 1) * 512],
                    start=(kt == 0), stop=(kt == 3))
            hs = slice(n2 * 512, (n2 + 1) * 512)
            cp = nc.scalar.copy if n2 == 1 else nc.vector.tensor_copy
            cp(so15[:, hs], pp[:])
            dqs[n2 % 3].dma_start(out_d[tsp15, hs], so15[:, hs])


_CACHE = {}


def _get_nc():
    if "nc" not in _CACHE:
        nc = bacc.Bacc("TRN2", target_bir_lowering=False, debug=False,
                       num_devices=N_CORES)
        build(nc)
        nc.compile()
        _CACHE["nc"] = nc
    return _CACHE["nc"]


def make_in_maps(x, w_attn, w_proj):
    x = np.asarray(x, dtype=np.float32)
    w_attn = np.asarray(w_attn, dtype=np.float32)
    w_proj = np.asarray(w_proj, dtype=np.float32)
    bf = ml_dtypes.bfloat16
    f8 = ml_dtypes.float8_e4m3
    in_maps = []
    for core in range(N_CORES):
        b, hg = divmod(core, 2)
        cs = slice(hg * GC, (hg + 1) * GC)
        xT = np.ascontiguousarray(x[b].T)
        in_maps.append({
            "xT8": xT.astype(f8),
            "xT": xT.astype(bf),
            "wq": np.ascontiguousarray(
                w_attn[:, 0 * C:1 * C][:, cs] * 32.0).astype(f8),
            "wk": np.ascontiguousarray(
                w_attn[:, 1 * C:2 * C][:, cs] * 32.0).astype(f8),
            "wv": np.ascontiguousarray(w_attn[:, 2 * C:3 * C][:, cs]).astype(bf),
            "wp": np.ascontiguousarray(w_proj[cs, :]).astype(bf),
        })
    return in_maps


def kernel(x, w_attn, w_proj, _trace=False, _trace_kwargs=None):
    nc = _get_nc()
    in_maps = make_in_maps(x, w_attn, w_proj)
    res = None
    for attempt in range(3):
        try:
            res = run_bass_kernel_spmd(nc, in_maps,
                                       core_ids=list(range(N_CORES)),
                                       trace=_trace, **(_trace_kwargs or {}))
            break
        except Exception:
            # a previous process can leave the device wedged
            # (NRT_EXEC_UNIT_UNRECOVERABLE); a retry recovers it
            if attempt == 2:
                raise
    _CACHE["last_results"] = res
    B = np.asarray(x).shape[0]
    out = np.empty((B, T, C), dtype=np.float32)
    for b in range(B):
        out[b] = (res.results[2 * b]["out"].astype(np.float32)
                  + res.results[2 * b + 1]["out"].astype(np.float32))
    return out
